# revision 50
# baseline (speedup 1.0000x reference)
"""Trainium2 Bass kernel for nn_PredicateTensorModel.

Math (reference):
  subj/verb/obj[c,d] = weighted embedding bags (N=8 ids per batch row)
  A[c,p,q]  = sum_i w[i,p,q] verb[c,i]
  US[c,p,q] = sum_j u[j,p,q] subj[c,j]
  out[c,q]  = sum_p US[c,p,q] * A[c,p,q] * obj[c,p]

Sharding: tensor-parallel over trailing q axis (32 q's per core, 8 cores).

Host prep is layout only (gather emb rows per id, transpose/cast w,u,
build the sparse bag-weight matrices S); every FLOP of the model runs on
device:
  - bag weighted-sums:   V.T @ S matmuls        (PE)
  - A and US:            two big matmuls        (PE, bf16, f32 PSUM)
  - US*obj fold:         tensor_tensor, split between Pool/gpsimd and
                         DVE (3 of 5 iterations fully on Pool) after Act
                         stages psU out of PSUM (gpsimd has no PSUM port)
  - A*(US*obj) + sum_p:  scalar_tensor_tensor with accum_out (DVE),
                         reading psA directly from PSUM

Scheduling notes (cost-model driven):
  - every DMA costs ~625ns of serialized HWDGE issue and +900ns sem
    propagation -> consts are batched into 2 loads, gathered rows are
    split per (tensor, c-block quarter) so each section unblocks its
    bag matmuls as soon as its bytes land, u-halves of the w/u tiles
    land before their w-halves (psU is consumed first)
  - emission interleaves the first 16 iterations with the bag quarters
    (the 6MB gather stream is the startup bottleneck)
  - PSUM: all 8 banks in one rotation tag (4 iterations in flight);
    psU is produced before psA each iteration since psA lives longest
  - dummy matmuls keep the PE p-state ramped while gathers land
  - output flushed in two chunks so the final DMA only covers q24..31
"""

import os
import sys

sys.path.insert(0, "/opt/trn_rl_repo")

import numpy as np
import ml_dtypes

N_CORES = 8
VOCAB, D, B, N = 50000, 256, 512, 8
QS = D // N_CORES  # 32 q columns per core
NQJ = QS // 2  # 16 q-pairs
NCHUNK = B // 16  # 32 gather chunks of 16 batch rows

bf16 = ml_dtypes.bfloat16

_PROG_CACHE = {}


def _build_program():
    import concourse.bass as bass
    import concourse.tile as tile
    import concourse.mybir as mybir
    from concourse import bacc
    from contextlib import ExitStack

    dt = mybir.dt
    nc = bacc.Bacc()

    # gathered embedding rows: [128, quarter, tensor (v,s,o), 2048]
    G_p = nc.declare_dram_parameter("G", [128, 4, 3, 2048], dt.bfloat16, isOutput=False)
    # consts: ident | S_v first (warmup + v bags), then S_s | S_o
    swarm_p = nc.declare_dram_parameter("swarm", [128, 128 + B], dt.bfloat16, isOutput=False)
    sso_p = nc.declare_dram_parameter("sso", [128, 2 * B], dt.bfloat16, isOutput=False)
    # w/u tiles: [128 i', qj, (t, ic, qq, d)] = [128, 16, 2048]
    wu_p = nc.declare_dram_parameter("wu", [128, NQJ, 2048], dt.bfloat16, isOutput=False)
    out_p = nc.declare_dram_parameter("out", [B, QS], dt.float32, isOutput=True)

    with ExitStack() as ctx:
        tc = ctx.enter_context(tile.TileContext(nc))
        const_pool = ctx.enter_context(tc.tile_pool(name="const", bufs=1))
        stage_pool = ctx.enter_context(tc.tile_pool(name="stg", bufs=16))
        junk_pool = ctx.enter_context(tc.tile_pool(name="junk", bufs=8))
        # 8 PSUM banks: 7 rotating (tag "mm") + 1 aux (warmup target, then
        # transpose staging)
        psum_pool = ctx.enter_context(tc.tile_pool(name="ps", bufs=7, space="PSUM"))

        # ---- constant loads (SP SEQ order == DMA issue order) ----
        swarm = const_pool.tile([128, 128 + B], dt.bfloat16, name="swarm", tag="swarm")
        nc.sync.dma_start(out=swarm[:], in_=swarm_p[:])
        ident = swarm[:, 0:128]
        sso = const_pool.tile([128, 2 * B], dt.bfloat16, name="sso", tag="sso")
        St = {"v": swarm[:, 128 : 128 + B], "s": sso[:, 0:B], "o": sso[:, B : 2 * B]}

        QW = NCHUNK * D // 4  # 2048 elements per G quarter
        TI = {"v": 0, "s": 1, "o": 2}

        Gq = {t: [] for t in "vso"}

        def load_G(t, k):
            g = const_pool.tile([128, QW], dt.bfloat16, name=f"G{t}{k}", tag=f"G{t}{k}")
            nc.sync.dma_start(out=g[:], in_=G_p[:, k, TI[t], :])
            Gq[t].append(g)

        wu_t = []

        def load_wu(qj):
            # u-half first: psU's matmuls run first in each iteration
            w = const_pool.tile([128, 2048], dt.bfloat16, name=f"wu{qj}", tag=f"wu{qj}")
            nc.sync.dma_start(out=w[:, 1024:2048], in_=wu_p[:, qj, 1024:2048])
            nc.sync.dma_start(out=w[:, 0:1024], in_=wu_p[:, qj, 0:1024])
            wu_t.append(w)

        # DMA order: swarm, Gv0, sso, Gs0, wu0, Go0, then per quarter
        # [Gv_k, Gs_k, wu_k, Go_k], then wu4..15
        nc.sync.dma_start(out=sso[:], in_=sso_p[:])
        load_G("s", 0)
        load_G("v", 0)
        load_G("o", 0)
        load_wu(0)
        for k in range(1, 4):
            load_G("s", k)
            load_G("v", k)
            load_wu(k)
            load_G("o", k)
        for qj in range(4, NQJ):
            load_wu(qj)

        # ---- phase E: embedding bags -> transposed [d, c] bf16 tiles ----
        embT = {}
        for t in "vso":
            embT[t] = [
                const_pool.tile([128, B], dt.bfloat16, name=f"eT{t}{dh}", tag=f"eT{t}{dh}")
                for dh in range(2)
            ]
        # obj in [c, p] layout (fold in1, 256-wide per q)
        obj_s = [
            const_pool.tile([128, D], dt.bfloat16, name=f"objs{ck}", tag=f"objs{ck}")
            for ck in range(4)
        ]

        def warm(n):
            # keep the PE p-state ramped while gathers land; no reader
            ps_warm = psum_pool.tile([128, B], dt.float32, name="warm", tag="mm", bufs=8)
            for _ in range(n):
                nc.tensor.matmul(
                    out=ps_warm[:], lhsT=ident, rhs=St["v"], start=True, stop=True
                )

        def bags(t, k):
            # quarter k of tensor t covers chunks 8k..8k+8 == c-block k;
            # psq lives on a dedicated aux bank so bag staging never blocks
            # the iteration tiles' "mm" rotation (PE is in-order)
            for dh in range(2):
                psq = psum_pool.tile([128, 128], dt.float32, name=f"psq{t}{k}{dh}", tag="mm", bufs=8)
                for ci in range(8):
                    ck = 8 * k + ci
                    nc.tensor.matmul(
                        out=psq[:, ci * 16 : (ci + 1) * 16],
                        lhsT=Gq[t][k][:, ci * 256 + dh * 128 : ci * 256 + dh * 128 + 128],
                        rhs=St[t][:, ck * 16 : (ck + 1) * 16],
                        start=True,
                        stop=True,
                    )
                nc.scalar.copy(out=embT[t][dh][:, k * 128 : (k + 1) * 128], in_=psq[:])

        def bags_obj(k):
            bags("o", k)
            # transpose [d, c-block] -> [c-block, d] via PE, stage to obj_s
            for dh in range(2):
                ptr = psum_pool.tile([128, 128], dt.bfloat16, name="ptr", tag="mm", bufs=8)
                nc.tensor.transpose(
                    out=ptr[:],
                    in_=embT["o"][dh][:, k * 128 : (k + 1) * 128],
                    identity=ident,
                )
                nc.scalar.copy(out=obj_s[k][:, dh * 128 : (dh + 1) * 128], in_=ptr[:])

        # ---- phase M ----
        outs = const_pool.tile([128, 4 * QS], dt.float32, name="outs", tag="outs")

        def m_iter_pe(qj, ck):
            # psU first: its consumer chain (Act copy) starts sooner, and
            # psA (held in PSUM until the STTs) is produced as late as
            # possible -- shorter PSUM residency, deeper pipelining
            psU = psum_pool.tile([128, 512], dt.float32, name="psU", tag="mm", bufs=8)
            for ic in range(2):
                nc.tensor.matmul(
                    out=psU[:],
                    lhsT=embT["s"][ic][:, ck * 128 : (ck + 1) * 128],
                    rhs=wu_t[qj][:, 1024 + ic * 512 : 1024 + (ic + 1) * 512],
                    start=(ic == 0),
                    stop=(ic == 1),
                )
            psA = psum_pool.tile([128, 512], dt.float32, name="psA", tag="mm", bufs=8)
            for ic in range(2):
                nc.tensor.matmul(
                    out=psA[:],
                    lhsT=embT["v"][ic][:, ck * 128 : (ck + 1) * 128],
                    rhs=wu_t[qj][:, ic * 512 : (ic + 1) * 512],
                    start=(ic == 0),
                    stop=(ic == 1),
                )
            return psU, psA

        def m_iter_vec(qj, ck, psU, psA):
            # gpsimd has no PSUM port: Act stages psU to SBUF
            USs = stage_pool.tile([128, 512], dt.bfloat16, name="USs", tag="us")
            nc.scalar.copy(out=USs[:], in_=psU[:])
            # fold obj. DVE (STTs) is the pipeline pacer and Pool has
            # slack, so Pool takes the q0 half and DVE the q1 half, and
            # every 3rd iteration Pool takes both halves.
            USo = [
                stage_pool.tile([128, D], dt.bfloat16, name=f"USo{qq}", tag=f"uo{qq}")
                for qq in range(2)
            ]
            it_idx = qj * 4 + ck
            if it_idx % 5 >= 2:
                for qq in range(2):
                    nc.gpsimd.tensor_tensor(
                        out=USo[qq][:], in0=USs[:, qq * D : (qq + 1) * D],
                        in1=obj_s[ck][:], op=mybir.AluOpType.mult,
                    )
            else:
                nc.gpsimd.tensor_tensor(
                    out=USo[0][:], in0=USs[:, 0:D], in1=obj_s[ck][:], op=mybir.AluOpType.mult
                )
                nc.vector.tensor_tensor(
                    out=USo[1][:], in0=USs[:, D : 2 * D], in1=obj_s[ck][:], op=mybir.AluOpType.mult
                )
            # fused A*(US*obj) + sum_p per q column (DVE; psA read from PSUM);
            # q1 first: its fold is DVE-local, no cross-engine wait
            for qq in (1, 0):
                junk = junk_pool.tile([128, D], dt.bfloat16, name="junk", tag="jk")
                nc.vector.scalar_tensor_tensor(
                    out=junk[:],
                    in0=psA[:, qq * D : (qq + 1) * D],
                    scalar=1.0,
                    in1=USo[qq][:],
                    op0=mybir.AluOpType.mult,
                    op1=mybir.AluOpType.mult,
                    accum_out=outs[:, ck * QS + qj * 2 + qq : ck * QS + qj * 2 + qq + 1],
                )

        def m_iter(qj, ck):
            psU, psA = m_iter_pe(qj, ck)
            m_iter_vec(qj, ck, psU, psA)

        # emission interleaves early iterations with bag quarters so PE
        # starts as soon as quarter 0 + wu0 land. bags_obj(k)'s writes to
        # obj_s[k] must be emitted before any (.,ck=k) fold reads it, but
        # the PE half of an iteration can go ahead of the obj bags.
        warm(4)
        bags("s", 0)
        warm(3)
        bags("v", 0)
        warm(2)
        ps00 = m_iter_pe(0, 0)
        warm(3)
        bags_obj(0)
        m_iter_vec(0, 0, *ps00)
        bags("s", 1)
        bags("v", 1)
        ps01 = m_iter_pe(0, 1)
        ps10 = m_iter_pe(1, 0)
        bags_obj(1)
        m_iter_vec(0, 1, *ps01)
        m_iter_vec(1, 0, *ps10)
        m_iter(1, 1)
        bags("s", 2)
        bags("v", 2)
        ps02 = m_iter_pe(0, 2)
        ps20 = m_iter_pe(2, 0)
        bags_obj(2)
        m_iter_vec(0, 2, *ps02)
        m_iter_vec(2, 0, *ps20)
        for qj, ck in [(1, 2), (2, 1), (2, 2)]:
            m_iter(qj, ck)
        bags("s", 3)
        bags("v", 3)
        for qj, ck in [(3, 0), (3, 1), (3, 2)]:
            m_iter(qj, ck)
        bags_obj(3)
        for qj, ck in [(0, 3), (1, 3), (2, 3), (3, 3)]:
            m_iter(qj, ck)
        for qj in range(4, NQJ):
            for ck in range(4):
                m_iter(qj, ck)
            if qj == 11:
                # columns 0..23 are final: flush them while qj12-15 run
                nc.sync.dma_start(
                    out=out_p.rearrange("(a c) q -> c a q", a=4)[:, :, 0:24],
                    in_=outs.rearrange("c (a q) -> c a q", a=4)[:, :, 0:24],
                )

        nc.sync.dma_start(
            out=out_p.rearrange("(a c) q -> c a q", a=4)[:, :, 24:QS],
            in_=outs.rearrange("c (a q) -> c a q", a=4)[:, :, 24:QS],
        )

    nc.finalize()
    return nc


def _get_program():
    if "nc" not in _PROG_CACHE:
        _PROG_CACHE["nc"] = _build_program()
    return _PROG_CACHE["nc"]


def _host_prep(inputs):
    """Layout-only host prep. Returns list of per-core in_maps."""
    ids = {}
    wts = {}
    for t, idk, wk in (
        ("s", "subj_id", "subj_w"),
        ("v", "verb_id", "verb_w"),
        ("o", "obj_id", "obj_w"),
    ):
        ids[t] = np.asarray(inputs[idk]).astype(np.int64)
        wts[t] = np.asarray(inputs[wk]).astype(np.float32)

    emb = np.asarray(inputs["emb"], dtype=np.float32)
    w = np.asarray(inputs["w"], dtype=np.float32)
    u = np.asarray(inputs["u"], dtype=np.float32)

    emb_b = emb.astype(bf16)
    # [i, p, q] -> [i, q, p] contiguous
    wT = np.ascontiguousarray(w.transpose(0, 2, 1)).astype(bf16)
    uT = np.ascontiguousarray(u.transpose(0, 2, 1)).astype(bf16)

    G_m = {}
    S_m = {}
    for t in "svo":
        # partition p = (c % 16)*8 + n ; chunk ck = c // 16
        ids_r = np.ascontiguousarray(
            ids[t].reshape(NCHUNK, 16, 8).transpose(1, 2, 0).reshape(128, NCHUNK)
        )
        G_m[t] = np.ascontiguousarray(emb_b[ids_r].reshape(128, NCHUNK * D))
        Sm = np.zeros((16, 8, NCHUNK, 16), np.float32)
        wr = wts[t].reshape(NCHUNK, 16, 8).transpose(1, 2, 0)
        j = np.arange(16)
        Sm[j[:, None, None], np.arange(8)[None, :, None], np.arange(NCHUNK)[None, None, :], j[:, None, None]] = wr
        S_m[t] = np.ascontiguousarray(Sm.reshape(128, B)).astype(bf16)

    ident = np.eye(128, dtype=bf16)
    swarm = np.ascontiguousarray(np.concatenate([ident, S_m["v"]], axis=1))
    sso = np.ascontiguousarray(np.concatenate([S_m["s"], S_m["o"]], axis=1))
    # [128, quarter, tensor (v,s,o), 2048]
    QW = NCHUNK * D // 4
    G = np.empty((128, 4, 3, QW), bf16)
    for i, t in enumerate("vso"):
        G[:, :, i, :] = G_m[t].reshape(128, 4, QW)
    G = np.ascontiguousarray(G)

    in_maps = []
    for k in range(N_CORES):
        wk = wT[:, k * QS : (k + 1) * QS, :]  # [256, 32, 256]
        uk = uT[:, k * QS : (k + 1) * QS, :]
        # [i, q, p] -> [128 i', qj, (t, ic, qq, p)]
        w5 = wk.reshape(2, 128, NQJ, 2, D).transpose(1, 2, 0, 3, 4)
        u5 = uk.reshape(2, 128, NQJ, 2, D).transpose(1, 2, 0, 3, 4)
        wu = np.ascontiguousarray(
            np.stack([w5, u5], axis=2).reshape(128, NQJ, 2048)
        )
        in_maps.append({"wu": wu, "G": G, "swarm": swarm, "sso": sso})
    return in_maps


def kernel(**inputs) -> np.ndarray:
    from concourse.bass_utils import run_bass_kernel_spmd

    nc = _get_program()
    in_maps = _host_prep(inputs)
    trace = bool(int(os.environ.get("KTRACE", "0")))
    res = run_bass_kernel_spmd(
        nc, in_maps, core_ids=list(range(N_CORES)), trace=trace
    )
    if trace:
        _PROG_CACHE["last_result"] = res

    out = np.concatenate(
        [res.results[k]["out"].astype(np.float32) for k in range(N_CORES)], axis=1
    )
    return out
